# revision 1
# baseline (speedup 1.0000x reference)
"""MoE routing kernel for Trainium2 (8 NeuronCores, expert-parallel).

Problem: nn_MoDE_52140902973544 (moe_routing).
  x[4,2048,1024], router (8 experts, top-2, capacity 1024), 7 real experts
  with FFN H=1024 -> I=4096 -> H=1024 (relu), expert 7 = identity (noop).

Strategy:
  * Host: router forward + top-2 + capacity-limited dispatch (pure index
    math, order-based -> float-robust), gather dispatched tokens per
    expert transposed to [H, cap].
  * Device (SPMD over 8 cores): core e computes
        outT_e = (relu(disp_e @ Wi_e) @ Wo_e).T            # [H, cap]
    Core 7 duplicates core 0 (7 real experts); its output is ignored.
  * Host: combine via pure gathers (no scatter) + gate weights + noop path.
"""

import os
import sys

for _p in ("/opt/trn_rl_repo", "/opt/pypackages"):
    if _p not in sys.path:
        sys.path.append(_p)

import numpy as np

# ---- problem constants (hardcoded per contract) ----
B, S, H, I = 4, 2048, 1024, 4096
E = 8                 # experts incl. noop (last)
ER = E - 1            # real experts
TOP_K = 2
N_TOK = B * S         # 8192
CAP = 1024            # ceil(N_TOK / E * 1.0)
N_CORES = 8

P = 128               # partitions
KO = H // P           # 8   H chunks
IC = I // P           # 32  I chunks
NF = 512              # matmul free dim
NN = CAP // NF        # 2   cap tiles

# matmul operand dtype: "f32" (exact, 4x slow), "f32r" (fast, ~tf32),
# "bf16" (fast, needs host cast)
MM_DTYPE = os.environ.get("MOE_MM_DTYPE", "bf16")

_CACHE = {}


def _build_nc(mm_dtype: str, repeat: int = 1):
    """Build the single-core Bass program (SPMD across 8 cores).

    Layout: x [H,cap] and h [I,cap] stay SBUF-resident in bf16; Wi is
    loaded in 4 pieces which the Wo pieces rotate into (pool tag ring)
    as GEMM1 consumes them, so weight DMA fully overlaps compute and the
    SBUF footprint stays ~176 KB/partition.  GEMM2 accumulates the full
    I contraction in PSUM (4 banks live) and a single outbound DMA
    writes outT.
    """
    import concourse.bacc as bacc
    import concourse.mybir as mybir
    import concourse.tile as tile

    dt = mybir.dt
    assert mm_dtype == "bf16"
    DT = dt.bfloat16

    # Bacc (not raw Bass): its compile() pipeline splits multi-semaphore
    # waits into event-semaphore chains (TRN2 allows 1 wait/instruction)
    # and moves matmul waits onto ldweights.
    nc = bacc.Bacc("TRN2")
    xT = nc.declare_dram_parameter("xT", [H, CAP], DT, isOutput=False)
    wi = nc.declare_dram_parameter("wi", [H, I], DT, isOutput=False)
    wo = nc.declare_dram_parameter("wo", [I, H], DT, isOutput=False)
    outT = nc.declare_dram_parameter("outT", [H, CAP], dt.float32, isOutput=True)
    NPIECE = 4
    IPP = IC // NPIECE        # 8 i-chunks per wi piece

    with tile.TileContext(nc) as tc:
        from contextlib import ExitStack

        with ExitStack() as ctx:
            xpool = ctx.enter_context(tc.tile_pool(name="x", bufs=1))
            wpool = ctx.enter_context(tc.tile_pool(name="w", bufs=NPIECE))
            hpool = ctx.enter_context(tc.tile_pool(name="h", bufs=1))
            opool = ctx.enter_context(tc.tile_pool(name="o", bufs=1))
            ps1pool = ctx.enter_context(
                tc.tile_pool(name="ps1", bufs=4, space="PSUM"))
            ps2pool = ctx.enter_context(
                tc.tile_pool(name="ps2", bufs=1, space="PSUM"))

            x_sb = xpool.tile([P, KO, CAP], DT)
            nc.sync.dma_start(x_sb[:], xT.rearrange("(ko p) n -> p ko n", p=P))
            h_sb = hpool.tile([P, IC, CAP], DT)
            out_sb = opool.tile([P, KO, CAP], dt.float32)

            wi_r = wi.rearrange("(ko p) i -> p ko i", p=P)     # [128, 8, 4096]
            wo_r = wo.rearrange("(ki p) h -> p ki h", p=P)     # [128, 32, 1024]

          # repeat loop for timing experiments (R x the full compute)
          # fmt: off
          # noqa
            for _rep in range(repeat):
              wi_pieces = []
              for p_ in range(NPIECE):
                  wt = wpool.tile([P, KO, IPP * P], DT, tag="w", name=f"wi{p_}")
                  nc.sync.dma_start(
                      wt[:], wi_r[:, :, p_ * IPP * P:(p_ + 1) * IPP * P])
                  wi_pieces.append(wt)
              # ---- GEMM1: hT = relu(Wi.T @ X.T) ----
              wo_pieces = []
              HPP = H // NPIECE                                  # 256 H cols
              for p_ in range(NPIECE):
                  wt = wi_pieces[p_]
                  for ir in range(IPP):
                      i = p_ * IPP + ir
                      for n in range(NN):
                          pt = ps1pool.tile([P, NF], dt.float32, tag="ps1",
                                            name=f"ps1_{i}_{n}")
                          for k in range(KO):
                              nc.tensor.matmul(
                                  pt[:],
                                  wt[:, k, ir * P:(ir + 1) * P],
                                  x_sb[:, k, n * NF:(n + 1) * NF],
                                  start=(k == 0),
                                  stop=(k == KO - 1),
                              )
                          nc.vector.tensor_scalar_max(
                              h_sb[:, i, n * NF:(n + 1) * NF], pt[:], 0.0
                          )
                  # piece p_ fully consumed -> rotate the wo piece into its
                  # slot (waits only on the PE release; lane observed above)
                  wot = wpool.tile([P, IC, HPP], DT, tag="w", name=f"wo{p_}")
                  nc.sync.dma_start(
                      wot[:], wo_r[:, :, p_ * HPP:(p_ + 1) * HPP])
                  wo_pieces.append(wot)

              # ---- GEMM2: outT = Wo.T @ hT ----
              HGM = HPP // P                                     # 2 m per piece
              for g in range(NPIECE):
                  wt = wo_pieces[g]
                  pts = [
                      [
                          ps2pool.tile([P, NF], dt.float32, tag=f"ps2_{m}_{n}",
                                       name=f"ps2_{g}_{m}_{n}")
                          for n in range(NN)
                      ]
                      for m in range(HGM)
                  ]
                  for k in range(IC):
                      for m in range(HGM):
                          for n in range(NN):
                              nc.tensor.matmul(
                                  pts[m][n][:],
                                  wt[:, k, m * P:(m + 1) * P],
                                  h_sb[:, k, n * NF:(n + 1) * NF],
                                  start=(k == 0),
                                  stop=(k == IC - 1),
                              )
                  for m in range(HGM):
                      for n in range(NN):
                          nc.vector.tensor_copy(
                              out_sb[:, g * HGM + m, n * NF:(n + 1) * NF],
                              pts[m][n][:])

            # single outbound DMA (outT is tracked whole-tensor; multiple
            # writers would chain WAW waits across DMA lanes)
            nc.sync.dma_start(
                outT.rearrange("(ko p) n -> p ko n", p=P), out_sb[:])
    nc.compile()
    return nc


def _get_nc(mm_dtype: str):
    if mm_dtype not in _CACHE:
        _CACHE[mm_dtype] = _build_nc(mm_dtype)
    return _CACHE[mm_dtype]


def _routing(x_flat: np.ndarray, router_w: np.ndarray, router_b: np.ndarray):
    """Replicate the reference router bit-for-bit where possible (jax CPU),
    returning top-2 values/indices [N_TOK, 2] (fp32/int)."""
    try:
        import jax
        import jax.numpy as jnp

        cpu = jax.devices("cpu")[0]
        with jax.default_device(cpu):
            xj = jnp.asarray(x_flat.reshape(B, S, H))
            logits = jnp.einsum("bsh,eh->bse", xj, jnp.asarray(router_w)) \
                + jnp.asarray(router_b)
            wflat = jax.nn.softmax(logits, axis=-1).reshape(N_TOK, E)
            topv, topi = jax.lax.top_k(wflat, TOP_K)
            return np.asarray(topv), np.asarray(topi)
    except Exception:
        # numpy fallback (float64 logits for a stable ordering)
        logits = x_flat.astype(np.float64) @ router_w.astype(np.float64).T \
            + router_b.astype(np.float64)
        m = logits.max(axis=1, keepdims=True)
        ex = np.exp(logits - m)
        wflat = (ex / ex.sum(axis=1, keepdims=True)).astype(np.float32)
        topi = np.argsort(-wflat, axis=1, kind="stable")[:, :TOP_K]
        topv = np.take_along_axis(wflat, topi, axis=1)
        return topv, topi


def kernel(x, router_w, router_b, experts_inter, experts_out):
    from concourse.bass_utils import run_bass_kernel_spmd

    x = np.ascontiguousarray(np.asarray(x, dtype=np.float32))
    router_w = np.asarray(router_w, dtype=np.float32)
    router_b = np.asarray(router_b, dtype=np.float32)
    experts_inter = np.asarray(experts_inter, dtype=np.float32)
    experts_out = np.asarray(experts_out, dtype=np.float32)

    x_flat = x.reshape(N_TOK, H)
    topv, topi = _routing(x_flat, router_w, router_b)

    # ---- capacity-limited dispatch (exact reference order semantics) ----
    # mask[t, e] = 1 iff expert e is in token t's top-2
    mask = np.zeros((N_TOK, E), dtype=bool)
    rows = np.arange(N_TOK)
    mask[rows[:, None], topi] = True
    expert_mask = mask[:, :ER]                       # [N, 7]
    # pos[t, e] = rank of t among selectors of e (token order)
    pos = np.cumsum(expert_mask, axis=0, dtype=np.int32) - 1

    # per-expert dispatched token lists (first CAP in token order)
    disp_T = np.zeros((ER, H, CAP), dtype=np.float32)
    for e in range(ER):
        idx_e = np.nonzero(expert_mask[:, e])[0][:CAP]
        disp_T[e, :, :len(idx_e)] = x_flat[idx_e].T

    # ---- device: per-expert FFN ----
    mm_dtype = MM_DTYPE
    import ml_dtypes

    bf = lambda a: np.ascontiguousarray(a.astype(ml_dtypes.bfloat16))
    f32c = np.ascontiguousarray
    cast1 = bf                                    # x, wi
    cast2 = bf                                    # wo

    in_maps = []
    for c in range(N_CORES):
        e = c if c < ER else 0
        in_maps.append({
            "xT": cast1(disp_T[e]),
            "wi": cast1(experts_inter[e]),
            "wo": cast2(experts_out[e]),
        })

    nc = _get_nc(mm_dtype)
    trace = bool(int(os.environ.get("MOE_TRACE", "0")))
    res = run_bass_kernel_spmd(nc, in_maps, list(range(N_CORES)), trace=trace)
    global LAST_RESULT
    LAST_RESULT = res
    out_T = np.stack([res.results[e]["outT"] for e in range(ER)])  # [7,H,cap]

    # ---- host combine: pure gathers ----
    out_flat = np.ascontiguousarray(out_T.transpose(0, 2, 1)).reshape(
        ER * CAP, H)
    out_ext = np.vstack([out_flat, np.zeros((1, H), dtype=np.float32)])

    combined = np.zeros_like(x_flat)
    noop_w = np.zeros(N_TOK, dtype=np.float32)
    for k in range(TOP_K):
        e_k = topi[:, k]
        v_k = topv[:, k]
        is_noop = e_k == ER
        noop_w += np.where(is_noop, v_k, 0.0).astype(np.float32)
        p_k = pos[rows, np.minimum(e_k, ER - 1)]
        ok = (~is_noop) & (p_k < CAP)
        slot = np.where(ok, np.minimum(e_k, ER - 1) * CAP + p_k, ER * CAP)
        combined += out_ext[slot] * np.where(ok, v_k, 0.0)[:, None]
    combined += x_flat * noop_w[:, None]

    return combined.reshape(B, S, H)



# revision 2
# speedup vs baseline: 296.4591x; 296.4591x over previous
"""MoE routing kernel for Trainium2 (8 NeuronCores, balanced expert-parallel).

Problem: nn_MoDE_52140902973544 (moe_routing).
  x[4,2048,1024], router (8 experts, top-2, capacity 1024), 7 real experts
  with FFN H=1024 -> I=4096 -> H=1024 (relu), expert 7 = identity (noop).

Strategy:
  * Host: router forward + top-2 + capacity-limited dispatch (pure index
    math, order-based -> float-robust), gather dispatched tokens per
    expert transposed to [H, cap].
  * Device (SPMD over 8 cores, I-dim split for load balance): each
    expert's FFN is split along the intermediate dim I=4096 into 8
    slices of 512.  Core e<7 runs slices 0-6 (I cols 0:3584) of expert
    e; core 7 runs slice 7 (I cols 3584:4096) of all 7 experts.  Every
    core executes the same program: 7 blocks, block g computing the
    partial  outP[g] = Wo_blk.T @ relu(Wi_blk.T @ xT[g])  with cores
    0-6 replicating their x across g.  896 N=512 matmuls/core
    (15.0 GFLOP) vs 1024 (17.2) for the naive one-expert-per-core
    split with an idle 8th core.
  * Host: sum the bf16 partials in fp32, combine via pure gathers
    (no scatter) + gate weights + noop path.
"""

import os
import sys

for _p in ("/opt/trn_rl_repo", "/opt/pypackages"):
    if _p not in sys.path:
        sys.path.append(_p)

import numpy as np

# ---- problem constants (hardcoded per contract) ----
B, S, H, I = 4, 2048, 1024, 4096
E = 8                 # experts incl. noop (last)
ER = E - 1            # real experts
TOP_K = 2
N_TOK = B * S         # 8192
CAP = 1024            # ceil(N_TOK / E * 1.0)
N_CORES = 8

P = 128               # partitions
KO = H // P           # 8  H-chunks
NBLK = 7              # I-slice blocks per core
IS = I // 8           # 512 I cols per block
ICB = IS // P         # 4  i-chunks per block
NF = 512              # matmul free dim
NN = CAP // NF        # 2  cap tiles
ITOT = NBLK * IS      # 3584 I cols per core

_CACHE = {}


def build_nc(repeat: int = 1, loop: int = 1):
    """Build the single-core Bass program (SPMD across 8 cores).

    ``repeat`` python-unrolls the whole 7-block body; ``loop`` wraps it
    in a hardware For_i loop (used only for timing: the program is
    idempotent, so looping it re-computes the same outputs while
    amortizing dispatch overhead).
    """
    import concourse.bacc as bacc
    import concourse.mybir as mybir
    import concourse.tile as tile

    dt = mybir.dt
    DT = dt.bfloat16

    # Bacc (not raw Bass): its compile() pipeline splits multi-semaphore
    # waits into event-semaphore chains (TRN2 allows 1 wait/instruction)
    # and moves matmul waits onto ldweights.
    nc = bacc.Bacc("TRN2")
    xT = nc.declare_dram_parameter("xT", [NBLK * H, CAP], DT, isOutput=False)
    wi = nc.declare_dram_parameter("wi", [H, ITOT], DT, isOutput=False)
    wo = nc.declare_dram_parameter("wo", [ITOT, H], DT, isOutput=False)
    outP = nc.declare_dram_parameter("outP", [NBLK * H, CAP], DT, isOutput=True)

    xT_r = xT.rearrange("(g ko p) n -> p (g ko) n", p=P, g=NBLK)   # [128,56,1024]
    wi_r = wi.rearrange("(ko p) i -> p ko i", p=P)                 # [128,8,3584]
    wo_r = wo.rearrange("(ki p) h -> p ki h", p=P)                 # [128,28,1024]
    outP_r = outP.rearrange("(g ko p) n -> p (g ko) n", p=P, g=NBLK)

    with tile.TileContext(nc) as tc:
        from contextlib import ExitStack

        with ExitStack() as ctx:
            xpool = ctx.enter_context(tc.tile_pool(name="x", bufs=2))
            wipool = ctx.enter_context(tc.tile_pool(name="wi", bufs=2))
            wopool = ctx.enter_context(tc.tile_pool(name="wo", bufs=2))
            hpool = ctx.enter_context(tc.tile_pool(name="h", bufs=2))
            opool = ctx.enter_context(tc.tile_pool(name="o", bufs=4))
            ps1pool = ctx.enter_context(
                tc.tile_pool(name="ps1", bufs=4, space="PSUM"))
            ps2pool = ctx.enter_context(
                tc.tile_pool(name="ps2", bufs=2, space="PSUM"))

            def body(rep):
                for g in range(NBLK):
                    # ---- prefetch block inputs (chunked for fast start) ----
                    x_t = xpool.tile([P, KO, CAP], DT, tag="x", name=f"x{rep}_{g}")
                    wi_t = wipool.tile([P, KO, IS], DT, tag="wi",
                                       name=f"wi{rep}_{g}")
                    # wi chunk 0 on the ACT queue (parallel with x chunk 0 on
                    # SP); block 0 also alternates x chunks across queues so
                    # arrivals outpace PE consumption during the cold start.
                    first = rep == 0 and g == 0
                    nc.scalar.dma_start(
                        wi_t[:, :, 0:P], wi_r[:, :, g * IS:g * IS + P])
                    for ko in range(KO):
                        eng = nc.scalar if (first and ko % 2 == 1) else nc.sync
                        eng.dma_start(
                            x_t[:, ko:ko + 1, :],
                            xT_r[:, g * KO + ko:g * KO + ko + 1, :])
                    for c in range(1, ICB):
                        nc.sync.dma_start(
                            wi_t[:, :, c * P:(c + 1) * P],
                            wi_r[:, :, g * IS + c * P:g * IS + (c + 1) * P])
                    wo_t = wopool.tile([P, ICB, H], DT, tag="wo",
                                       name=f"wo{rep}_{g}")
                    nc.sync.dma_start(
                        wo_t[:], wo_r[:, g * ICB:(g + 1) * ICB, :])

                    # ---- GEMM1: h = relu(Wi_blk.T @ x)  [IS, CAP] ----
                    # k-outer tile groups.  Block 0 uses groups of 4 so every
                    # arriving x k-chunk feeds 4 matmuls during the cold
                    # start; later blocks use groups of 2 (ps1 ring distance
                    # 2) so a group's first matmul never waits on the
                    # previous group's relu.
                    h_t = hpool.tile([P, ICB, CAP], DT, tag="h",
                                     name=f"h{rep}_{g}")
                    gsz = 4 if first else 2
                    groups = [[(t // NN, t % NN)
                               for t in range(g0, g0 + gsz)]
                              for g0 in range(0, ICB * NN, gsz)]
                    for tiles in groups:
                        pts1 = {t: ps1pool.tile([P, NF], dt.float32, tag="ps1",
                                                name=f"g1_{rep}_{g}_{t[0]}_{t[1]}")
                                for t in tiles}
                        for k in range(KO):
                            for (i, n) in tiles:
                                nc.tensor.matmul(
                                    pts1[(i, n)][:],
                                    wi_t[:, k, i * P:(i + 1) * P],
                                    x_t[:, k, n * NF:(n + 1) * NF],
                                    start=(k == 0),
                                    stop=(k == KO - 1),
                                )
                        for (i, n) in tiles:
                            nc.vector.tensor_scalar_max(
                                h_t[:, i, n * NF:(n + 1) * NF],
                                pts1[(i, n)][:], 0.0)

                    # ---- GEMM2: outP[g] = Wo_blk.T @ h  [H, CAP] ----
                    # per-m groups (8 per block); ps2 double-buffered so the
                    # next group's matmuls never wait on this group's copies.
                    for m in range(KO):
                        o_t = opool.tile([P, 1, CAP], DT, tag="o",
                                         name=f"o{rep}_{g}_{m}")
                        pts = [ps2pool.tile([P, NF], dt.float32,
                                            tag=f"ps2_{n}",
                                            name=f"g2_{rep}_{g}_{m}_{n}")
                               for n in range(NN)]
                        for k in range(ICB):
                            for n in range(NN):
                                nc.tensor.matmul(
                                    pts[n][:],
                                    wo_t[:, k, m * P:(m + 1) * P],
                                    h_t[:, k, n * NF:(n + 1) * NF],
                                    start=(k == 0),
                                    stop=(k == ICB - 1),
                                )
                        for n in range(NN):
                            nc.vector.tensor_copy(
                                o_t[:, 0, n * NF:(n + 1) * NF], pts[n][:])
                        nc.scalar.dma_start(
                            outP_r[:, g * KO + m:g * KO + m + 1, :], o_t[:])

            if loop > 1:
                with tc.For_i(0, loop, 1):
                    for rep in range(repeat):
                        body(rep)
            else:
                for rep in range(repeat):
                    body(rep)
    nc.compile()
    return nc


def _get_nc():
    if "v2" not in _CACHE:
        _CACHE["v2"] = build_nc()
    return _CACHE["v2"]


def _routing(x_flat: np.ndarray, router_w: np.ndarray, router_b: np.ndarray):
    """Replicate the reference router bit-for-bit where possible (jax CPU),
    returning top-2 values/indices [N_TOK, 2] (fp32/int)."""
    try:
        import jax
        import jax.numpy as jnp

        cpu = jax.devices("cpu")[0]
        with jax.default_device(cpu):
            xj = jnp.asarray(x_flat.reshape(B, S, H))
            logits = jnp.einsum("bsh,eh->bse", xj, jnp.asarray(router_w)) \
                + jnp.asarray(router_b)
            wflat = jax.nn.softmax(logits, axis=-1).reshape(N_TOK, E)
            topv, topi = jax.lax.top_k(wflat, TOP_K)
            return np.asarray(topv), np.asarray(topi)
    except Exception:
        # numpy fallback (float64 logits for a stable ordering)
        logits = x_flat.astype(np.float64) @ router_w.astype(np.float64).T \
            + router_b.astype(np.float64)
        m = logits.max(axis=1, keepdims=True)
        ex = np.exp(logits - m)
        wflat = (ex / ex.sum(axis=1, keepdims=True)).astype(np.float32)
        topi = np.argsort(-wflat, axis=1, kind="stable")[:, :TOP_K]
        topv = np.take_along_axis(wflat, topi, axis=1)
        return topv, topi


def _dispatch(x_flat, topi):
    """Capacity-limited dispatch: per-expert first-CAP tokens in token
    order, gathered transposed to [ER, H, CAP]; also the per-token rank."""
    mask = np.zeros((N_TOK, E), dtype=bool)
    rows = np.arange(N_TOK)
    mask[rows[:, None], topi] = True
    expert_mask = mask[:, :ER]                       # [N, 7]
    pos = np.cumsum(expert_mask, axis=0, dtype=np.int32) - 1
    disp_T = np.zeros((ER, H, CAP), dtype=np.float32)
    for e in range(ER):
        idx_e = np.nonzero(expert_mask[:, e])[0][:CAP]
        disp_T[e, :, :len(idx_e)] = x_flat[idx_e].T
    return disp_T, pos, rows


def make_in_maps(disp_T, experts_inter, experts_out):
    """Per-core device inputs for the I-split SPMD program."""
    import ml_dtypes

    bf = lambda a: np.ascontiguousarray(a.astype(ml_dtypes.bfloat16))
    in_maps = []
    for e in range(ER):
        in_maps.append({
            "xT": bf(np.tile(disp_T[e], (NBLK, 1))),
            "wi": bf(experts_inter[e][:, :ITOT]),
            "wo": bf(experts_out[e][:ITOT, :]),
        })
    in_maps.append({
        "xT": bf(disp_T.reshape(ER * H, CAP)),
        "wi": bf(np.concatenate(
            [experts_inter[e][:, ITOT:] for e in range(ER)], axis=1)),
        "wo": bf(np.concatenate(
            [experts_out[e][ITOT:, :] for e in range(ER)], axis=0)),
    })
    return in_maps


def combine_partials(results):
    """Sum the per-core bf16 partials into per-expert [ER, H, CAP] fp32."""
    parts = [np.asarray(results[c]["outP"]).astype(np.float32)
             .reshape(NBLK, H, CAP) for c in range(N_CORES)]
    out_T = np.empty((ER, H, CAP), dtype=np.float32)
    for e in range(ER):
        out_T[e] = parts[e].sum(axis=0) + parts[ER][e]
    return out_T


def kernel(x, router_w, router_b, experts_inter, experts_out):
    from concourse.bass_utils import run_bass_kernel_spmd

    x = np.ascontiguousarray(np.asarray(x, dtype=np.float32))
    router_w = np.asarray(router_w, dtype=np.float32)
    router_b = np.asarray(router_b, dtype=np.float32)
    experts_inter = np.asarray(experts_inter, dtype=np.float32)
    experts_out = np.asarray(experts_out, dtype=np.float32)

    x_flat = x.reshape(N_TOK, H)
    topv, topi = _routing(x_flat, router_w, router_b)
    disp_T, pos, rows = _dispatch(x_flat, topi)

    in_maps = make_in_maps(disp_T, experts_inter, experts_out)
    nc = _get_nc()
    res = run_bass_kernel_spmd(nc, in_maps, list(range(N_CORES)))
    global LAST_RESULT
    LAST_RESULT = res
    out_T = combine_partials(res.results)            # [7, H, cap]

    # ---- host combine: pure gathers ----
    out_flat = np.ascontiguousarray(out_T.transpose(0, 2, 1)).reshape(
        ER * CAP, H)
    out_ext = np.vstack([out_flat, np.zeros((1, H), dtype=np.float32)])

    combined = np.zeros_like(x_flat)
    noop_w = np.zeros(N_TOK, dtype=np.float32)
    for k in range(TOP_K):
        e_k = topi[:, k]
        v_k = topv[:, k]
        is_noop = e_k == ER
        noop_w += np.where(is_noop, v_k, 0.0).astype(np.float32)
        p_k = pos[rows, np.minimum(e_k, ER - 1)]
        ok = (~is_noop) & (p_k < CAP)
        slot = np.where(ok, np.minimum(e_k, ER - 1) * CAP + p_k, ER * CAP)
        combined += out_ext[slot] * np.where(ok, v_k, 0.0)[:, None]
    combined += x_flat * noop_w[:, None]

    return combined.reshape(B, S, H)
